# revision 38
# baseline (speedup 1.0000x reference)
"""Trainium2 Bass kernel for DepthWiseSeparableAttention.

Reference computation (B=1, N=4096, C=256, HEADS=8, HEAD_DIM=32):
    xn   = LayerNorm(x)
    qkv  = BatchNorm_eval(xn @ w_qkv.T + b_qkv)          -> q, k, v  [B,h,N,d]
    attn = q @ k.T * d^-0.5                              [B,h,N,N]
    bias = depthwise_conv7x7(mean_keys(attn))            [B,h,N,1]  (per-query)
    out  = softmax(attn + bias) @ v                      [B,h,N,d]
    out  = x + (out @ w_proj.T + b_proj)

Key identity: `bias` is constant along the softmax (key) axis, and softmax is
shift-invariant, so the entire mean->conv->bias path cancels exactly.  The
kernel therefore computes plain attention; LN gain/bias and the eval-mode BN
are folded into the qkv weights on the host.

Sharding: heads-parallel, 1 head per NeuronCore (8 cores).  Each core runs
LayerNorm + its head's qkv projection + attention + its slice of the output
projection, producing a [N, C] partial.  Host unshard = sum of partials
+ x + b_proj.

Device layout per core (flash-style over key tiles):
    xnT  [128, 2, N]  : LayerNorm(x) transposed (c on partitions, 2 halves)
    qT   [32, N]      : queries, head dim on partitions
    kvT  [64, N]      : keys (rows 0:32, ST stationary) + values (rows 32:64,
                        transposed into von via PE row-group 1)
    von  [128, nk, 33]: per key-tile [V_kt | 1] (keys on partitions)
    per q-chunk (512 queries):
        ST[key,q] psum <- kT_kt.T @ qT_chunk  (pairs of key tiles share one
                                               [128,1024] psum for a single
                                               wide exp on ScalarE)
        E = exp(scale * ST)
        OT[33, 512] psum += von_kt.T @ E      (rows 0-31 = V.T@E, row 32 =
                                               colsum for softmax denom)
        proj: PT = w_projT.T @ OT, transpose back to [tok, c], scale by
              1/colsum (softmax denom commutes with the projection).
"""

import numpy as np

# ---- problem constants (hardcoded; kernel.py must be self-contained) ----
N_TOK = 4096
C = 256
HEADS = 8
D = 32
LN_EPS = 1e-6
BN_EPS = 1e-5
SCALE = D ** -0.5
N_CORES = 8

# matmul dtype mode: "f32" (exact, 4 cyc/row), "f32r" (1 cyc/row), "bf16"
MM_MODE = "f32r"
TRACE = False
LAST_RESULTS = None  # BassKernelResults of the last run (for test.py)

_NC_CACHE = {}


def build_nc(n_tok=N_TOK, mm=MM_MODE):
    """Build the single-core Bass program (SPMD across 8 cores via inputs)."""
    from contextlib import ExitStack

    import concourse.mybir as mybir
    import concourse.tile as tile
    from concourse import bacc
    from concourse.masks import make_identity

    f32 = mybir.dt.float32
    # matmul-feeding tiles use this dtype; producers (DVE/ACT) round on write
    mdt = {
        "f32": f32,
        "f32r": mybir.dt.float32r,
        "bf16": mybir.dt.bfloat16,
    }[mm]

    assert n_tok % 512 == 0
    nt = n_tok // 128   # token tiles / out tiles
    nk = n_tok // 128   # key tiles
    nq = n_tok // 512   # q-chunks
    ng = n_tok // 512   # qkv projection groups
    npair = nk // 2

    AF = mybir.ActivationFunctionType
    ALU = mybir.AluOpType

    nc = bacc.Bacc()
    x_d = nc.declare_dram_parameter("x", [n_tok, C], f32, False)
    wq_d = nc.declare_dram_parameter("wqT", [128, 2, D], mdt, False)
    wkv_d = nc.declare_dram_parameter("wkvT", [128, 2, 2 * D], mdt, False)
    bq_d = nc.declare_dram_parameter("bq", [D, 1], f32, False)
    bkv_d = nc.declare_dram_parameter("bkv", [2 * D, 1], f32, False)
    wp_d = nc.declare_dram_parameter("wprojT", [D, C], mdt, False)
    out_d = nc.declare_dram_parameter("partial", [n_tok, C], f32, True)

    with tile.TileContext(nc) as tc, ExitStack() as ctx:
        consts = ctx.enter_context(tc.tile_pool(name="consts", bufs=1))
        work = ctx.enter_context(tc.tile_pool(name="work", bufs=5))
        stats = ctx.enter_context(tc.tile_pool(name="stats", bufs=8))
        big = ctx.enter_context(tc.tile_pool(name="big", bufs=1))
        epool = ctx.enter_context(tc.tile_pool(name="epool", bufs=4))
        otsb = ctx.enter_context(tc.tile_pool(name="otsb", bufs=3))
        ptp = ctx.enter_context(tc.tile_pool(name="ptp", bufs=3))
        outp = ctx.enter_context(tc.tile_pool(name="outp", bufs=3))
        ps_small = ctx.enter_context(
            tc.tile_pool(name="ps_small", bufs=2, space="PSUM")
        )
        # qkv (phase 1) and OT accumulators (phase 2) share one 2-slot pool:
        # same bank budget, but adjacent q-chunks get distinct OT banks so
        # the next chunk's PV needn't wait for the previous OT's copy-out
        ps_acc = ctx.enter_context(tc.tile_pool(name="ps_acc", bufs=2, space="PSUM"))
        ps_st = ctx.enter_context(tc.tile_pool(name="ps_st", bufs=2, space="PSUM"))

        # ---- constants ----
        ident = consts.tile([128, 128], f32)
        make_identity(nc, ident)
        eps_t = consts.tile([128, 1], f32)
        nc.vector.memset(eps_t, LN_EPS)
        # weights go through the gpsimd (SWDGE) queue to keep the HWDGE ring
        # free for the bulk x/out traffic
        wq_sb = consts.tile([128, 2, D], mdt)
        nc.gpsimd.dma_start(out=wq_sb, in_=wq_d[:, :, :])
        wkv_sb = consts.tile([128, 2, 2 * D], mdt)
        nc.gpsimd.dma_start(out=wkv_sb, in_=wkv_d[:, :, :])
        bq_sb = consts.tile([D, 1], f32)
        nc.gpsimd.dma_start(out=bq_sb, in_=bq_d[:, :])
        bkv_sb = consts.tile([2 * D, 1], f32)
        nc.gpsimd.dma_start(out=bkv_sb, in_=bkv_d[:, :])
        wp_sb = consts.tile([D, C], mdt)
        nc.gpsimd.dma_start(out=wp_sb, in_=wp_d[:, :])

        # ---- persistent big tiles ----
        xnT = big.tile([128, 2, n_tok], mdt)
        qT = big.tile([D, n_tok], mdt)
        kvT = big.tile([2 * D, n_tok], mdt)
        von = big.tile([128, nk, D + 1], mdt)
        recipT = big.tile([128, nt], f32)
        ones_t = consts.tile([128, nk], f32)
        nc.vector.memset(ones_t, 1.0)
        nc.vector.tensor_copy(out=von[:, :, D], in_=ones_t)

        # ---- phase 1: LayerNorm + transpose ----
        NB = 4  # token tiles per x DMA (batch DMAs: per-op overhead dominates)
        x_batched = x_d[:, :].rearrange("(b a p) c -> b p a c", a=NB, p=128)
        for ib in range(nt // NB):
            xb = work.tile([128, NB, C], f32, tag="x_t")
            nc.sync.dma_start(out=xb, in_=x_batched[ib])
            mvb = stats.tile([128, NB, 2], f32, tag="mv")
            for j in range(NB):
                st6 = stats.tile([128, 6], f32, tag="st6")
                nc.vector.bn_stats(out=st6, in_=xb[:, j, :])
                nc.vector.bn_aggr(out=mvb[:, j, :], in_=st6)
            # one batched sqrt(var+eps) for the whole group (ACT access
            # latency dominates small ops)
            lvb = stats.tile([128, NB], f32, tag="sd")
            nc.scalar.activation(out=lvb, in_=mvb[:, :, 1], func=AF.Ln, bias=eps_t)
            rstdb = stats.tile([128, NB], f32, tag="rstd")
            nc.scalar.activation(out=rstdb, in_=lvb, func=AF.Exp, scale=-0.5)
            for j in range(NB):
                i = ib * NB + j
                xn = work.tile([128, C], f32, tag="xn")
                # gpsimd (Pool) is otherwise idle in phase 1
                nc.gpsimd.tensor_scalar(
                    out=xn,
                    in0=xb[:, j, :],
                    scalar1=mvb[:, j, 0:1],
                    scalar2=rstdb[:, j : j + 1],
                    op0=ALU.subtract,
                    op1=ALU.mult,
                )
                tp = ps_small.tile([128, 2, 128], f32, tag="ps_small")
                for half in (0, 1):
                    nc.tensor.transpose(
                        tp[:, half, :], xn[:, half * 128 : (half + 1) * 128], ident
                    )
                # single fused copy of both halves on ScalarE (idle early)
                nc.scalar.copy(out=xnT[:, :, i * 128 : (i + 1) * 128], in_=tp)

        # ---- phase 1b: qkv projection (per 512-token group) + V transpose ----
        for g in range(ng):
            sl = slice(g * 512, (g + 1) * 512)
            for wsb, bsb, dstT, m in (
                (wq_sb, bq_sb, qT, D),
                (wkv_sb, bkv_sb, kvT, 2 * D),
            ):
                ps = ps_acc.tile([2 * D, 512], f32, tag="acc")
                nc.tensor.matmul(
                    ps[:m, :], wsb[:, 0, :], xnT[:, 0, sl], start=True, stop=False
                )
                nc.tensor.matmul(
                    ps[:m, :], wsb[:, 1, :], xnT[:, 1, sl], start=False, stop=True
                )
                nc.vector.tensor_scalar_add(
                    out=dstT[:, sl], in0=ps[:m, :], scalar1=bsb
                )
            for j in range(4):
                kt = g * 4 + j
                tpv = ps_small.tile([128, D], f32, tag="ps_small")
                # v rows live at partitions 32:64 of kvT; PE row-group 1 is
                # addressed by slicing the identity at the same base partition
                nc.tensor.transpose(
                    tpv,
                    kvT[D : 2 * D, kt * 128 : (kt + 1) * 128].bitcast(f32),
                    ident[D : 2 * D, D : 2 * D],
                )
                nc.vector.tensor_copy(out=von[:, kt, 0:D], in_=tpv)

        # ---- phase 2: attention per q-chunk ----
        # The epilogue for chunk qc is emitted AFTER chunk qc+1's attention
        # loop: its PE work (proj + transposes) then fills PE idle slots while
        # ScalarE (the phase-2 bottleneck) stays saturated with exps.
        out_batched = out_d[:, :].rearrange("(b a p) c -> b p a c", a=4, p=128)

        def epilogue(qc, ot_sb):
            # colsum -> partition 0 -> transpose -> reciprocal
            cs = otsb.tile([1, 512], f32, tag="cs")
            nc.gpsimd.dma_start(out=cs, in_=ot_sb[D : D + 1, :].bitcast(f32))
            for c4 in range(4):
                tcs = ps_small.tile([128, 1], f32, tag="ps_small")
                nc.tensor.transpose(
                    tcs, cs[0:1, c4 * 128 : (c4 + 1) * 128], ident[0:1, 0:1]
                )
                t_idx = qc * 4 + c4
                nc.vector.reciprocal(out=recipT[:, t_idx : t_idx + 1], in_=tcs)
            # output projection (on unnormalized OT; denom applied at the end)
            pt = []
            for mh in (0, 1):
                pj = ps_small.tile([128, 512], f32, tag="ps_small")
                nc.tensor.matmul(
                    pj,
                    wp_sb[:, mh * 128 : (mh + 1) * 128],
                    ot_sb[0:D, :],
                    start=True,
                    stop=True,
                )
                pt_sb = ptp.tile([128, 512], f32, tag="pt")
                nc.vector.tensor_copy(out=pt_sb, in_=pj)
                pt.append(pt_sb)
            ob = outp.tile([128, 4, C], f32, tag="o_t")
            for c4 in range(4):
                t_idx = qc * 4 + c4
                for mh in (0, 1):
                    tpp = ps_small.tile([128, 128], f32, tag="ps_small")
                    nc.tensor.transpose(
                        tpp, pt[mh][:, c4 * 128 : (c4 + 1) * 128], ident
                    )
                    nc.vector.tensor_scalar_mul(
                        out=ob[:, c4, mh * 128 : (mh + 1) * 128],
                        in0=tpp,
                        scalar1=recipT[:, t_idx : t_idx + 1],
                    )
            nc.sync.dma_start(out=out_batched[qc], in_=ob)

        pending = None  # (qc, ot_sb) awaiting epilogue
        for qc in range(nq):
            qsl = slice(qc * 512, (qc + 1) * 512)
            ot_acc = ps_acc.tile([2 * D, 512], f32, tag="acc")
            ot_ps = ot_acc[: D + 1, :]
            for p in range(npair):
                st = ps_st.tile([128, 1024], f32, tag="st")
                for j in (0, 1):
                    kt = p * 2 + j
                    nc.tensor.matmul(
                        st[:, j * 512 : (j + 1) * 512],
                        kvT[0:D, kt * 128 : (kt + 1) * 128],
                        qT[:, qsl],
                        start=True,
                        stop=True,
                    )
                e = epool.tile([128, 1024], mdt)
                nc.scalar.activation(out=e, in_=st, func=AF.Exp, scale=SCALE)
                for j in (0, 1):
                    kt = p * 2 + j
                    nc.tensor.matmul(
                        ot_ps,
                        von[:, kt, :],
                        e[:, j * 512 : (j + 1) * 512],
                        start=(kt == 0),
                        stop=(kt == nk - 1),
                    )
            ot_sb = otsb.tile([D + 1, 512], mdt)
            nc.vector.tensor_copy(out=ot_sb, in_=ot_ps)
            if pending is not None:
                epilogue(*pending)
            pending = (qc, ot_sb)
        epilogue(*pending)

    nc.compile()
    return nc


def fold_weights(ln_g, ln_b, w_qkv, b_qkv, bn_g, bn_b, bn_mean, bn_var):
    """Fold LayerNorm gain/bias + eval-mode BatchNorm into qkv weight/bias."""
    s = bn_g / np.sqrt(bn_var + BN_EPS)
    W3 = w_qkv * ln_g[None, :] * s[:, None]
    b3 = (b_qkv + w_qkv @ ln_b - bn_mean) * s + bn_b
    return W3.astype(np.float32), b3.astype(np.float32)


def _wT_head(W3, base, h):
    """[256, 32] slice for head h transposed into device layout [128, 2, 32]."""
    w = W3[base + h * D : base + (h + 1) * D, :]  # [32, 256]
    wT = np.ascontiguousarray(w.T.reshape(2, 128, D).transpose(1, 0, 2))
    return wT.astype(np.float32)


def kernel(**inputs):
    from concourse.bass_utils import run_bass_kernel_spmd

    global LAST_RESULTS

    x = np.asarray(inputs["x"], dtype=np.float32)
    B = x.shape[0]
    x2 = x.reshape(N_TOK, C)
    ln_g = np.asarray(inputs["ln_g"], dtype=np.float32)
    ln_b = np.asarray(inputs["ln_b"], dtype=np.float32)
    w_qkv = np.asarray(inputs["w_qkv"], dtype=np.float32)
    b_qkv = np.asarray(inputs["b_qkv"], dtype=np.float32)
    bn_g = np.asarray(inputs["bn_g"], dtype=np.float32)
    bn_b = np.asarray(inputs["bn_b"], dtype=np.float32)
    bn_mean = np.asarray(inputs["bn_mean"], dtype=np.float32)
    bn_var = np.asarray(inputs["bn_var"], dtype=np.float32)
    w_proj = np.asarray(inputs["w_proj"], dtype=np.float32)
    b_proj = np.asarray(inputs["b_proj"], dtype=np.float32)

    W3, b3 = fold_weights(ln_g, ln_b, w_qkv, b_qkv, bn_g, bn_b, bn_mean, bn_var)

    if MM_MODE not in _NC_CACHE:
        _NC_CACHE[MM_MODE] = build_nc(N_TOK, MM_MODE)
    nc = _NC_CACHE[MM_MODE]

    in_maps = []
    for h in range(N_CORES):
        bq = b3[h * D : (h + 1) * D]
        bk = b3[C + h * D : C + (h + 1) * D]
        bv = b3[2 * C + h * D : 2 * C + (h + 1) * D]
        in_maps.append(
            {
                "x": x2,
                "wqT": _wT_head(W3, 0, h),
                "wkvT": np.concatenate(
                    [_wT_head(W3, C, h), _wT_head(W3, 2 * C, h)], axis=2
                ),
                "bq": bq[:, None].astype(np.float32),
                "bkv": np.concatenate([bk, bv])[:, None].astype(np.float32),
                "wprojT": np.ascontiguousarray(
                    w_proj[:, h * D : (h + 1) * D].T, dtype=np.float32
                ),
            }
        )

    res = run_bass_kernel_spmd(
        nc, in_maps, core_ids=list(range(N_CORES)), trace=TRACE
    )
    LAST_RESULTS = res
    partial = res.results[0]["partial"].astype(np.float32).copy()
    for r in res.results[1:]:
        partial += r["partial"]
    out = x2 + b_proj[None, :] + partial
    return out.reshape(B, N_TOK, C).astype(np.float32)


# revision 45
# speedup vs baseline: 1.0243x; 1.0243x over previous
"""Trainium2 Bass kernel for DepthWiseSeparableAttention.

Reference computation (B=1, N=4096, C=256, HEADS=8, HEAD_DIM=32):
    xn   = LayerNorm(x)
    qkv  = BatchNorm_eval(xn @ w_qkv.T + b_qkv)          -> q, k, v  [B,h,N,d]
    attn = q @ k.T * d^-0.5                              [B,h,N,N]
    bias = depthwise_conv7x7(mean_keys(attn))            [B,h,N,1]  (per-query)
    out  = softmax(attn + bias) @ v                      [B,h,N,d]
    out  = x + (out @ w_proj.T + b_proj)

Key identity: `bias` is constant along the softmax (key) axis, and softmax is
shift-invariant, so the entire mean->conv->bias path cancels exactly.  The
kernel therefore computes plain attention; LN gain/bias and the eval-mode BN
are folded into the qkv weights on the host.

Sharding: heads-parallel, 1 head per NeuronCore (8 cores).  Each core runs
LayerNorm + its head's qkv projection + attention + its slice of the output
projection, producing a [N, C] partial.  Host unshard = sum of partials
+ x + b_proj.

Device layout per core (flash-style over key tiles):
    xnT  [128, 2, N]  : LayerNorm(x) transposed (c on partitions, 2 halves)
    qT   [32, N]      : queries, head dim on partitions
    kvT  [64, N]      : keys (rows 0:32, ST stationary) + values (rows 32:64,
                        transposed into von via PE row-group 1)
    von  [128, nk, 33]: per key-tile [V_kt | 1] (keys on partitions)
    per q-chunk (512 queries):
        ST[key,q] psum <- kT_kt.T @ qT_chunk  (pairs of key tiles share one
                                               [128,1024] psum for a single
                                               wide exp on ScalarE)
        E = exp(scale * ST)
        OT[33, 512] psum += von_kt.T @ E      (rows 0-31 = V.T@E, row 32 =
                                               colsum for softmax denom)
        proj: PT = w_projT.T @ OT, transpose back to [tok, c], scale by
              1/colsum (softmax denom commutes with the projection).
"""

import numpy as np

# ---- problem constants (hardcoded; kernel.py must be self-contained) ----
N_TOK = 4096
C = 256
HEADS = 8
D = 32
LN_EPS = 1e-6
BN_EPS = 1e-5
SCALE = D ** -0.5
N_CORES = 8

# matmul dtype mode: "f32" (exact, 4 cyc/row), "f32r" (1 cyc/row), "bf16"
MM_MODE = "f32r"
TRACE = False
LAST_RESULTS = None  # BassKernelResults of the last run (for test.py)

_NC_CACHE = {}


def build_nc(n_tok=N_TOK, mm=MM_MODE):
    """Build the single-core Bass program (SPMD across 8 cores via inputs)."""
    from contextlib import ExitStack

    import concourse.mybir as mybir
    import concourse.tile as tile
    from concourse import bacc
    from concourse.masks import make_identity

    f32 = mybir.dt.float32
    # matmul-feeding tiles use this dtype; producers (DVE/ACT) round on write
    mdt = {
        "f32": f32,
        "f32r": mybir.dt.float32r,
        "bf16": mybir.dt.bfloat16,
    }[mm]

    assert n_tok % 512 == 0
    nt = n_tok // 128   # token tiles / out tiles
    nk = n_tok // 128   # key tiles
    nq = n_tok // 512   # q-chunks
    ng = n_tok // 512   # qkv projection groups
    npair = nk // 2

    AF = mybir.ActivationFunctionType
    ALU = mybir.AluOpType

    nc = bacc.Bacc()
    x_d = nc.declare_dram_parameter("x", [n_tok, C], f32, False)
    wq_d = nc.declare_dram_parameter("wqT", [128, 2, D], mdt, False)
    wkv_d = nc.declare_dram_parameter("wkvT", [128, 2, 2 * D], mdt, False)
    bq_d = nc.declare_dram_parameter("bq", [D, 1], f32, False)
    bkv_d = nc.declare_dram_parameter("bkv", [2 * D, 1], f32, False)
    wp_d = nc.declare_dram_parameter("wprojT", [D, C], mdt, False)
    out_d = nc.declare_dram_parameter("partial", [n_tok, C], f32, True)

    with tile.TileContext(nc) as tc, ExitStack() as ctx:
        consts = ctx.enter_context(tc.tile_pool(name="consts", bufs=1))
        work = ctx.enter_context(tc.tile_pool(name="work", bufs=5))
        stats = ctx.enter_context(tc.tile_pool(name="stats", bufs=8))
        big = ctx.enter_context(tc.tile_pool(name="big", bufs=1))
        epool = ctx.enter_context(tc.tile_pool(name="epool", bufs=4))
        otsb = ctx.enter_context(tc.tile_pool(name="otsb", bufs=3))
        ptp = ctx.enter_context(tc.tile_pool(name="ptp", bufs=3))
        outp = ctx.enter_context(tc.tile_pool(name="outp", bufs=3))
        ps_small = ctx.enter_context(
            tc.tile_pool(name="ps_small", bufs=2, space="PSUM")
        )
        # qkv (phase 1) and OT accumulators (phase 2) share one 2-slot pool:
        # same bank budget, but adjacent q-chunks get distinct OT banks so
        # the next chunk's PV needn't wait for the previous OT's copy-out
        ps_acc = ctx.enter_context(tc.tile_pool(name="ps_acc", bufs=2, space="PSUM"))
        ps_st = ctx.enter_context(tc.tile_pool(name="ps_st", bufs=2, space="PSUM"))

        # ---- constants ----
        ident = consts.tile([128, 128], f32)
        make_identity(nc, ident)
        eps_t = consts.tile([128, 1], f32)
        nc.vector.memset(eps_t, LN_EPS)
        # weights go through the gpsimd (SWDGE) queue to keep the HWDGE ring
        # free for the bulk x/out traffic
        wq_sb = consts.tile([128, 2, D], mdt)
        nc.gpsimd.dma_start(out=wq_sb, in_=wq_d[:, :, :])
        wkv_sb = consts.tile([128, 2, 2 * D], mdt)
        nc.gpsimd.dma_start(out=wkv_sb, in_=wkv_d[:, :, :])
        bq_sb = consts.tile([D, 1], f32)
        nc.gpsimd.dma_start(out=bq_sb, in_=bq_d[:, :])
        bkv_sb = consts.tile([2 * D, 1], f32)
        nc.gpsimd.dma_start(out=bkv_sb, in_=bkv_d[:, :])
        wp_sb = consts.tile([D, C], mdt)
        nc.gpsimd.dma_start(out=wp_sb, in_=wp_d[:, :])

        # ---- persistent big tiles ----
        xnT = big.tile([128, 2, n_tok], mdt)
        qT = big.tile([D, n_tok], mdt)
        kvT = big.tile([2 * D, n_tok], mdt)
        von = big.tile([128, nk, D + 1], mdt)
        recipT = big.tile([128, nt], f32)
        ones_t = consts.tile([128, nk], f32)
        nc.vector.memset(ones_t, 1.0)
        nc.vector.tensor_copy(out=von[:, :, D], in_=ones_t)

        # ---- phase 1: LayerNorm + transpose ----
        NB = 4  # token tiles per x DMA (batch DMAs: per-op overhead dominates)
        x_batched = x_d[:, :].rearrange("(b a p) c -> b p a c", a=NB, p=128)
        for ib in range(nt // NB):
            xb = work.tile([128, NB, C], f32, tag="x_t")
            nc.sync.dma_start(out=xb, in_=x_batched[ib])
            mvb = stats.tile([128, NB, 2], f32, tag="mv")
            for j in range(NB):
                st6 = stats.tile([128, 6], f32, tag="st6")
                nc.vector.bn_stats(out=st6, in_=xb[:, j, :])
                nc.vector.bn_aggr(out=mvb[:, j, :], in_=st6)
            # one batched sqrt(var+eps) for the whole group (ACT access
            # latency dominates small ops)
            lvb = stats.tile([128, NB], f32, tag="sd")
            nc.scalar.activation(out=lvb, in_=mvb[:, :, 1], func=AF.Ln, bias=eps_t)
            rstdb = stats.tile([128, NB], f32, tag="rstd")
            nc.scalar.activation(out=rstdb, in_=lvb, func=AF.Exp, scale=-0.5)
            for j in range(NB):
                i = ib * NB + j
                xn = work.tile([128, C], f32, tag="xn")
                # gpsimd (Pool) is otherwise idle in phase 1
                nc.gpsimd.tensor_scalar(
                    out=xn,
                    in0=xb[:, j, :],
                    scalar1=mvb[:, j, 0:1],
                    scalar2=rstdb[:, j : j + 1],
                    op0=ALU.subtract,
                    op1=ALU.mult,
                )
                tp = ps_small.tile([128, 2, 128], f32, tag="ps_small")
                for half in (0, 1):
                    nc.tensor.transpose(
                        tp[:, half, :], xn[:, half * 128 : (half + 1) * 128], ident
                    )
                # single fused copy of both halves on ScalarE (idle early)
                nc.scalar.copy(out=xnT[:, :, i * 128 : (i + 1) * 128], in_=tp)

        # ---- phase 1b: qkv projection (per 512-token group) + V transpose ----
        for g in range(ng):
            sl = slice(g * 512, (g + 1) * 512)
            for wsb, bsb, dstT, m in (
                (wq_sb, bq_sb, qT, D),
                (wkv_sb, bkv_sb, kvT, 2 * D),
            ):
                ps = ps_acc.tile([2 * D, 512], f32, tag="acc")
                nc.tensor.matmul(
                    ps[:m, :], wsb[:, 0, :], xnT[:, 0, sl], start=True, stop=False
                )
                nc.tensor.matmul(
                    ps[:m, :], wsb[:, 1, :], xnT[:, 1, sl], start=False, stop=True
                )
                nc.vector.tensor_scalar_add(
                    out=dstT[:, sl], in0=ps[:m, :], scalar1=bsb
                )
            for j in range(4):
                kt = g * 4 + j
                tpv = ps_small.tile([128, D], f32, tag="ps_small")
                # v rows live at partitions 32:64 of kvT; PE row-group 1 is
                # addressed by slicing the identity at the same base partition
                nc.tensor.transpose(
                    tpv,
                    kvT[D : 2 * D, kt * 128 : (kt + 1) * 128].bitcast(f32),
                    ident[D : 2 * D, D : 2 * D],
                )
                nc.vector.tensor_copy(out=von[:, kt, 0:D], in_=tpv)

        # ---- phase 2: attention per q-chunk ----
        # The epilogue for chunk qc is emitted AFTER chunk qc+1's attention
        # loop: its PE work (proj + transposes) then fills PE idle slots while
        # ScalarE (the phase-2 bottleneck) stays saturated with exps.
        out_batched = out_d[:, :].rearrange("(b a p) c -> b p a c", a=4, p=128)

        def epilogue(qc, ot_sb):
            # colsum lives on partition 32 (last OT row); transpose it
            # straight from there via PE row-group 1 (identity sliced at the
            # same base partition), 4 column-chunks into one psum tile, then
            # one batched reciprocal
            tcs = ps_small.tile([128, 4], f32, tag="ps_small")
            for c4 in range(4):
                nc.tensor.transpose(
                    tcs[:, c4 : c4 + 1],
                    ot_sb[D : D + 1, c4 * 128 : (c4 + 1) * 128].bitcast(f32),
                    ident[D : D + 1, D : D + 1],
                )
            nc.vector.reciprocal(
                out=recipT[:, qc * 4 : (qc + 1) * 4], in_=tcs
            )
            # output projection (on unnormalized OT; denom applied at the end)
            pt = []
            for mh in (0, 1):
                pj = ps_small.tile([128, 512], f32, tag="ps_small")
                nc.tensor.matmul(
                    pj,
                    wp_sb[:, mh * 128 : (mh + 1) * 128],
                    ot_sb[0:D, :],
                    start=True,
                    stop=True,
                )
                pt_sb = ptp.tile([128, 512], f32, tag="pt")
                nc.vector.tensor_copy(out=pt_sb, in_=pj)
                pt.append(pt_sb)
            ob = outp.tile([128, 4, C], f32, tag="o_t")
            for c4 in range(4):
                t_idx = qc * 4 + c4
                tpp = ps_small.tile([128, 2, 128], f32, tag="ps_small")
                for mh in (0, 1):
                    nc.tensor.transpose(
                        tpp[:, mh, :], pt[mh][:, c4 * 128 : (c4 + 1) * 128], ident
                    )
                # both dout halves share the token's softmax denom -> one op
                nc.vector.tensor_scalar_mul(
                    out=ob[:, c4, :],
                    in0=tpp,
                    scalar1=recipT[:, t_idx : t_idx + 1],
                )
            nc.sync.dma_start(out=out_batched[qc], in_=ob)

        pending = None  # (qc, ot_sb) awaiting epilogue
        for qc in range(nq):
            qsl = slice(qc * 512, (qc + 1) * 512)
            ot_acc = ps_acc.tile([2 * D, 512], f32, tag="acc")
            ot_ps = ot_acc[: D + 1, :]
            for p in range(npair):
                st = ps_st.tile([128, 1024], f32, tag="st")
                for j in (0, 1):
                    kt = p * 2 + j
                    nc.tensor.matmul(
                        st[:, j * 512 : (j + 1) * 512],
                        kvT[0:D, kt * 128 : (kt + 1) * 128],
                        qT[:, qsl],
                        start=True,
                        stop=True,
                    )
                e = epool.tile([128, 1024], mdt)
                nc.scalar.activation(out=e, in_=st, func=AF.Exp, scale=SCALE)
                for j in (0, 1):
                    kt = p * 2 + j
                    nc.tensor.matmul(
                        ot_ps,
                        von[:, kt, :],
                        e[:, j * 512 : (j + 1) * 512],
                        start=(kt == 0),
                        stop=(kt == nk - 1),
                    )
            ot_sb = otsb.tile([D + 1, 512], mdt)
            nc.vector.tensor_copy(out=ot_sb, in_=ot_ps)
            if pending is not None:
                epilogue(*pending)
            pending = (qc, ot_sb)
        epilogue(*pending)

    nc.compile()
    return nc


def fold_weights(ln_g, ln_b, w_qkv, b_qkv, bn_g, bn_b, bn_mean, bn_var):
    """Fold LayerNorm gain/bias + eval-mode BatchNorm into qkv weight/bias."""
    s = bn_g / np.sqrt(bn_var + BN_EPS)
    W3 = w_qkv * ln_g[None, :] * s[:, None]
    b3 = (b_qkv + w_qkv @ ln_b - bn_mean) * s + bn_b
    return W3.astype(np.float32), b3.astype(np.float32)


def _wT_head(W3, base, h):
    """[256, 32] slice for head h transposed into device layout [128, 2, 32]."""
    w = W3[base + h * D : base + (h + 1) * D, :]  # [32, 256]
    wT = np.ascontiguousarray(w.T.reshape(2, 128, D).transpose(1, 0, 2))
    return wT.astype(np.float32)


def kernel(**inputs):
    from concourse.bass_utils import run_bass_kernel_spmd

    global LAST_RESULTS

    x = np.asarray(inputs["x"], dtype=np.float32)
    B = x.shape[0]
    x2 = x.reshape(N_TOK, C)
    ln_g = np.asarray(inputs["ln_g"], dtype=np.float32)
    ln_b = np.asarray(inputs["ln_b"], dtype=np.float32)
    w_qkv = np.asarray(inputs["w_qkv"], dtype=np.float32)
    b_qkv = np.asarray(inputs["b_qkv"], dtype=np.float32)
    bn_g = np.asarray(inputs["bn_g"], dtype=np.float32)
    bn_b = np.asarray(inputs["bn_b"], dtype=np.float32)
    bn_mean = np.asarray(inputs["bn_mean"], dtype=np.float32)
    bn_var = np.asarray(inputs["bn_var"], dtype=np.float32)
    w_proj = np.asarray(inputs["w_proj"], dtype=np.float32)
    b_proj = np.asarray(inputs["b_proj"], dtype=np.float32)

    W3, b3 = fold_weights(ln_g, ln_b, w_qkv, b_qkv, bn_g, bn_b, bn_mean, bn_var)

    if MM_MODE not in _NC_CACHE:
        _NC_CACHE[MM_MODE] = build_nc(N_TOK, MM_MODE)
    nc = _NC_CACHE[MM_MODE]

    in_maps = []
    for h in range(N_CORES):
        bq = b3[h * D : (h + 1) * D]
        bk = b3[C + h * D : C + (h + 1) * D]
        bv = b3[2 * C + h * D : 2 * C + (h + 1) * D]
        in_maps.append(
            {
                "x": x2,
                "wqT": _wT_head(W3, 0, h),
                "wkvT": np.concatenate(
                    [_wT_head(W3, C, h), _wT_head(W3, 2 * C, h)], axis=2
                ),
                "bq": bq[:, None].astype(np.float32),
                "bkv": np.concatenate([bk, bv])[:, None].astype(np.float32),
                "wprojT": np.ascontiguousarray(
                    w_proj[:, h * D : (h + 1) * D].T, dtype=np.float32
                ),
            }
        )

    res = run_bass_kernel_spmd(
        nc, in_maps, core_ids=list(range(N_CORES)), trace=TRACE
    )
    LAST_RESULTS = res
    partial = res.results[0]["partial"].astype(np.float32).copy()
    for r in res.results[1:]:
        partial += r["partial"]
    out = x2 + b_proj[None, :] + partial
    return out.reshape(B, N_TOK, C).astype(np.float32)
